# revision 1
# baseline (speedup 1.0000x reference)
"""ChildSumTreeLSTM on 8 trn2 NeuronCores.

Tree is a reversed complete 4-ary heap (id = N-1-heap, heap j's children are
4j+1..4j+4).  Shard the 64 depth-3 subtrees rooted at heap 21..84 contiguously
across 8 cores (8 subtrees/core).  Each core runs a uniform padded forest
(levels of 512/128/32/8 slots) with zero-padded slots; a leaf is identical to
an internal node whose children rows are zero, so one level-step kernel serves
everything.  One 32KB AllGather moves the 64 subtree roots everywhere, then
every core redundantly computes the 21-node top tree (heap 0..20) and writes
the root h.

All on-device tensors use "T layout": mem dim (512 -> 4 partition tiles of
128) on partitions, node slots on the free dim.  GEMMs are out.T = W.T @ actT
with weight k-tiles as the stationary operand.
"""

import os
import sys

sys.path.insert(0, "/opt/trn_rl_repo")

import numpy as np

import concourse.bass as bass
import concourse.bacc as bacc
import concourse.mybir as mybir
import concourse.tile as tile
from concourse.bass_utils import run_bass_kernel_spmd

F32 = mybir.dt.float32
BF16 = mybir.dt.float16  # GEMM operand dtype (fp16: single-pass PE, 10-bit mantissa)
AF = mybir.ActivationFunctionType
ALU = mybir.AluOpType
AX = mybir.AxisListType

N = 4096
MEM = 512
IN_DIM = 512
NCORES = 8
P = 128
KT = 4  # contraction tiles (512 / 128)

# slot layout in the 704-column per-core node array
NL3, NL2, NL1, NL0 = 512, 128, 32, 8
OFF3, OFF2, OFF1, OFF0 = 0, 512, 640, 672
OFFT2, OFFT1, OFFT0 = 680, 696, 700
NSLOT = 704
NHALF = 352

TOP_ON_DEVICE = os.environ.get("KERNEL_TOP", "device") == "device"

LAST_RESULT = None  # BassKernelResults of the most recent run (for test.py)


def _core_heaps(c):
    t0 = 21 + 8 * c
    heaps = []
    for s in range(8):
        heaps += [64 * (t0 + s) + 21 + a for a in range(64)]  # rel3
    for s in range(8):
        heaps += [16 * (t0 + s) + 5 + a for a in range(16)]  # rel2
    for s in range(8):
        heaps += [4 * (t0 + s) + 1 + a for a in range(4)]  # rel1
    for s in range(8):
        heaps += [t0 + s]  # rel0
    heaps += list(range(5, 21)) + list(range(1, 5)) + [0]  # T2, T1, T0
    heaps += [-1, -1, -1]  # pad to 704
    return np.array(heaps, dtype=np.int64)


def _build_program():
    nc = bacc.Bacc("TRN2", target_bir_lowering=False, debug=False)

    xin_d = nc.dram_tensor("xin", [IN_DIM, NSLOT], BF16, kind="ExternalInput")
    wx_d = nc.dram_tensor("wx", [IN_DIM, 4 * MEM], BF16, kind="ExternalInput")
    ws_d = nc.dram_tensor("ws", [MEM, 3 * MEM], BF16, kind="ExternalInput")
    wf_d = nc.dram_tensor("wf", [MEM, MEM], BF16, kind="ExternalInput")
    bx_d = nc.dram_tensor("bx", [P, 16], F32, kind="ExternalInput")
    bs_d = nc.dram_tensor("bs", [P, 12], F32, kind="ExternalInput")
    bf_d = nc.dram_tensor("bf", [P, 4], F32, kind="ExternalInput")
    cm_d = nc.dram_tensor("cmask", [P, NL3], F32, kind="ExternalInput")
    out_d = nc.dram_tensor("out", [1, MEM], F32, kind="ExternalOutput")
    if not TOP_ON_DEVICE:
        roots_d = nc.dram_tensor("roots", [2 * MEM, NL0], F32, kind="ExternalOutput")
    if TOP_ON_DEVICE:
        contrib_d = nc.dram_tensor("contrib", [2 * MEM, NL0], F32)
        gath_d = nc.dram_tensor("gath", [NCORES * 2 * MEM, NL0], F32,
                                addr_space="Shared")

    with tile.TileContext(nc) as tc:
        with (
            tc.tile_pool(name="wpool", bufs=1) as wpool,
            tc.tile_pool(name="xpool", bufs=1) as xpool,
            tc.tile_pool(name="state", bufs=1) as state,
            tc.tile_pool(name="tmp", bufs=3) as tmp,
            tc.tile_pool(name="psA", bufs=3, space="PSUM") as psA,
            tc.tile_pool(name="psB", bufs=2, space="PSUM") as psB,
            tc.tile_pool(name="psF", bufs=2, space="PSUM") as psF,
        ):
            # ---- load everything ----
            wx_s = [wpool.tile([P, 4 * MEM], BF16, name=f"t", tag=f"wx{k}") for k in range(KT)]
            ws_s = [wpool.tile([P, 3 * MEM], BF16, name=f"t", tag=f"ws{k}") for k in range(KT)]
            wf_s = [wpool.tile([P, MEM], BF16, name=f"t", tag=f"wf{k}") for k in range(KT)]
            in_s = [wpool.tile([P, NSLOT], BF16, name=f"t", tag=f"in{k}") for k in range(KT)]
            for k in range(KT):
                r = slice(k * P, (k + 1) * P)
                nc.sync.dma_start(wx_s[k][:], wx_d[r, :])
                nc.sync.dma_start(in_s[k][:], xin_d[r, :])
                nc.sync.dma_start(ws_s[k][:], ws_d[r, :])
                nc.sync.dma_start(wf_s[k][:], wf_d[r, :])
            bx_s = wpool.tile([P, 16], F32, name="t", tag="bx")
            bs_s = wpool.tile([P, 12], F32, name="t", tag="bs")
            bf_s = wpool.tile([P, 4], F32, name="t", tag="bf")
            cm_s = wpool.tile([P, NL3], F32, name="t", tag="cm")
            nc.sync.dma_start(bx_s[:], bx_d[:])
            nc.sync.dma_start(bs_s[:], bs_d[:])
            nc.sync.dma_start(bf_s[:], bf_d[:])
            nc.sync.dma_start(cm_s[:], cm_d[:])

            # ---- phase A: X.T[2048, 704] = Wx.T @ xin (+bx) ----
            Xt = [xpool.tile([P, NSLOT], F32, name=f"t", tag=f"X{mc}") for mc in range(16)]
            for mc in range(16):
                for h in range(2):
                    ncols = slice(h * NHALF, (h + 1) * NHALF)
                    ps = psA.tile([P, NHALF], F32, name="t", tag="psA")
                    for k in range(KT):
                        nc.tensor.matmul(
                            ps[:],
                            wx_s[k][:, mc * P:(mc + 1) * P],
                            in_s[k][:, ncols],
                            start=(k == 0),
                            stop=(k == KT - 1),
                        )
                    # copy psum->sbuf with the bias add fused in (on DVE)
                    nc.vector.tensor_scalar_add(
                        Xt[mc][:, ncols], ps[:], bx_s[:, mc:mc + 1]
                    )

            # ---- rel3 leaf step ----
            H3 = [state.tile([P, NL3], BF16, name=f"t", tag=f"H3{m}") for m in range(KT)]
            C3 = [state.tile([P, NL3], F32, name=f"t", tag=f"C3{m}") for m in range(KT)]
            for m in range(KT):
                ig = tmp.tile([P, NL3], F32, name="t", tag="lf_i")
                og = tmp.tile([P, NL3], F32, name="t", tag="lf_o")
                ug = tmp.tile([P, NL3], F32, name="t", tag="lf_u")
                nc.scalar.activation(ig[:], Xt[m][:, OFF3:OFF3 + NL3],
                                     AF.Sigmoid, bias=bs_s[:, m:m + 1])
                nc.scalar.activation(og[:], Xt[8 + m][:, OFF3:OFF3 + NL3],
                                     AF.Sigmoid, bias=bs_s[:, 4 + m:5 + m])
                nc.scalar.activation(ug[:], Xt[12 + m][:, OFF3:OFF3 + NL3],
                                     AF.Tanh, bias=bs_s[:, 8 + m:9 + m])
                cr = tmp.tile([P, NL3], F32, name="t", tag="lf_c")
                nc.vector.tensor_mul(cr[:], ig[:], ug[:])
                nc.vector.tensor_mul(C3[m][:], cr[:], cm_s[:])  # zero pad slots
                th = tmp.tile([P, NL3], F32, name="t", tag="lf_t")
                nc.scalar.activation(th[:], C3[m][:], AF.Tanh)
                nc.vector.tensor_mul(H3[m][:], og[:], th[:])

            def level_step(n_par, x_off, Hc, Cc, hname, h_dtype=BF16):
                """One ChildSumTreeLSTM level: parents at X cols
                [x_off, x_off+n_par), children tiles Hc/Cc [128, 4*n_par]."""
                nch = 4 * n_par
                # f = sigmoid(Wf.T @ Hc + fx + bf); fccs = sum_children f*cc
                fccs = []
                for m in range(KT):
                    ps = psF.tile([P, nch], F32, name="t", tag="psF")
                    for k in range(KT):
                        nc.tensor.matmul(
                            ps[:], wf_s[k][:, m * P:(m + 1) * P], Hc[k][:],
                            start=(k == 0), stop=(k == KT - 1),
                        )
                    tf = tmp.tile([P, nch], F32, name="t", tag="st_tf")
                    pv = ps[:].rearrange("p (n g) -> p n g", g=4)
                    tv = tf[:].rearrange("p (n g) -> p n g", g=4)
                    fx = Xt[4 + m][:, x_off:x_off + n_par]
                    fxb = bass.AP(tensor=fx.tensor, offset=fx.offset,
                                  ap=list(fx.ap) + [[0, 4]])
                    nc.vector.tensor_add(tv[:], pv[:], fxb)
                    fg = tmp.tile([P, nch], F32, name="t", tag="st_fg")
                    nc.scalar.activation(fg[:], tf[:], AF.Sigmoid,
                                         bias=bf_s[:, m:m + 1])
                    fcc = tmp.tile([P, nch], F32, name="t", tag="st_fcc")
                    nc.vector.tensor_mul(fcc[:], fg[:], Cc[m][:])
                    fs = tmp.tile([P, n_par], F32, name="t", tag="st_fs")
                    nc.vector.tensor_reduce(
                        fs[:], fcc[:].rearrange("p (n g) -> p n g", g=4),
                        axis=AX.X, op=ALU.add,
                    )
                    fccs.append(fs)
                # child-h sum (groups of 4 adjacent columns)
                chs = [tmp.tile([P, n_par], BF16, name=f"t", tag=f"chs{k}") for k in range(KT)]
                for k in range(KT):
                    chf = tmp.tile([P, n_par], F32, name="t", tag="chf")
                    nc.vector.tensor_reduce(
                        chf[:],
                        Hc[k][:].rearrange("p (n g) -> p n g", g=4),
                        axis=AX.X, op=ALU.add,
                    )
                    nc.vector.tensor_copy(chs[k][:], chf[:])
                # iou.T = Ws.T @ chs (+bs)
                iou = [tmp.tile([P, n_par], F32, name=f"t", tag=f"iou{mc}") for mc in range(12)]
                for mc in range(12):
                    ps = psB.tile([P, n_par], F32, name="t", tag="psB")
                    for k in range(KT):
                        nc.tensor.matmul(
                            ps[:], ws_s[k][:, mc * P:(mc + 1) * P], chs[k][:],
                            start=(k == 0), stop=(k == KT - 1),
                        )
                    nc.vector.tensor_scalar_add(iou[mc][:], ps[:],
                                                bs_s[:, mc:mc + 1])
                Hp, Cp = [], []
                for m in range(KT):
                    pi = tmp.tile([P, n_par], F32, name="t", tag="st_pi")
                    po = tmp.tile([P, n_par], F32, name="t", tag="st_po")
                    pu = tmp.tile([P, n_par], F32, name="t", tag="st_pu")
                    nc.vector.tensor_add(pi[:], Xt[m][:, x_off:x_off + n_par],
                                         iou[m][:])
                    nc.vector.tensor_add(po[:], Xt[8 + m][:, x_off:x_off + n_par],
                                         iou[4 + m][:])
                    nc.vector.tensor_add(pu[:], Xt[12 + m][:, x_off:x_off + n_par],
                                         iou[8 + m][:])
                    ig = tmp.tile([P, n_par], F32, name="t", tag="st_ig")
                    og = tmp.tile([P, n_par], F32, name="t", tag="st_og")
                    ug = tmp.tile([P, n_par], F32, name="t", tag="st_ug")
                    nc.scalar.activation(ig[:], pi[:], AF.Sigmoid)
                    nc.scalar.activation(og[:], po[:], AF.Sigmoid)
                    nc.scalar.activation(ug[:], pu[:], AF.Tanh)
                    cp = state.tile([P, n_par], F32, name=f"t", tag=f"C_{hname}{m}")
                    iu = tmp.tile([P, n_par], F32, name="t", tag="st_iu")
                    nc.vector.tensor_mul(iu[:], ig[:], ug[:])
                    nc.vector.tensor_add(cp[:], iu[:], fccs[m][:])
                    th = tmp.tile([P, n_par], F32, name="t", tag="st_th")
                    nc.scalar.activation(th[:], cp[:], AF.Tanh)
                    hp = state.tile([P, n_par], h_dtype, name=f"t", tag=f"H_{hname}{m}")
                    nc.vector.tensor_mul(hp[:], og[:], th[:])
                    Hp.append(hp)
                    Cp.append(cp)
                return Hp, Cp

            H2, C2 = level_step(NL2, OFF2, H3, C3, "L2")
            H1, C1 = level_step(NL1, OFF1, H2, C2, "L1")
            H0, C0 = level_step(NL0, OFF0, H1, C1, "L0")

            if TOP_ON_DEVICE:
                # gather the 64 subtree roots (h and c) to every core
                for m in range(KT):
                    h0f = tmp.tile([P, NL0], F32, name="t", tag="h0f")
                    nc.vector.tensor_copy(h0f[:], H0[m][:])
                    nc.sync.dma_start(contrib_d[m * P:(m + 1) * P, :], h0f[:])
                    nc.sync.dma_start(contrib_d[MEM + m * P:MEM + (m + 1) * P, :],
                                      C0[m][:])
                nc.gpsimd.collective_compute(
                    "AllGather", ALU.bypass,
                    replica_groups=[list(range(NCORES))],
                    ins=[contrib_d[:]],
                    outs=[gath_d[:]],
                )
                H64f = [state.tile([P, 64], F32, name=f"t", tag=f"H64f{m}") for m in range(KT)]
                H64 = [state.tile([P, 64], BF16, name=f"t", tag=f"H64{m}") for m in range(KT)]
                C64 = [state.tile([P, 64], F32, name=f"t", tag=f"C64{m}") for m in range(KT)]
                # gath rows: 1024*r + 512*hc + 128*m + p ; cols: 8 roots
                gv = gath_d[:].rearrange("(r hc m p) c -> hc m p r c",
                                         r=NCORES, hc=2, m=KT)
                for m in range(KT):
                    nc.sync.dma_start(
                        H64f[m][:].rearrange("p (r c) -> p r c", r=NCORES),
                        gv[0, m],
                    )
                    nc.vector.tensor_copy(H64[m][:], H64f[m][:])
                    nc.sync.dma_start(
                        C64[m][:].rearrange("p (r c) -> p r c", r=NCORES),
                        gv[1, m],
                    )
                HT2, CT2 = level_step(16, OFFT2, H64, C64, "T2")
                HT1, CT1 = level_step(4, OFFT1, HT2, CT2, "T1")
                HT0, _ = level_step(1, OFFT0, HT1, CT1, "T0", h_dtype=F32)
                for m in range(KT):
                    nc.sync.dma_start(out_d[0, m * P:(m + 1) * P], HT0[m][:])
            else:
                for m in range(KT):
                    nc.sync.dma_start(roots_d[m * P:(m + 1) * P, :], H0[m][:])
                    nc.sync.dma_start(roots_d[MEM + m * P:MEM + (m + 1) * P, :],
                                      C0[m][:])
                z = wpool.tile([P, 4], F32, name="t", tag="zero")
                nc.vector.memset(z[:], 0.0)
                nc.sync.dma_start(out_d[0, :].rearrange("(m p) -> p m", p=P), z[:])

    nc.compile()
    return nc


_NC_CACHE = None


def kernel(inputs, Wx, bx, Ws, bs, Wf, bf, children):
    global LAST_RESULT, _NC_CACHE
    inputs = np.asarray(inputs, np.float32)
    Wx = np.asarray(Wx, np.float32)
    bx = np.asarray(bx, np.float32)
    Ws = np.asarray(Ws, np.float32)
    bs = np.asarray(bs, np.float32)
    Wf = np.asarray(Wf, np.float32)
    bf = np.asarray(bf, np.float32)

    Wx_b = Wx.astype(np.float16)
    Ws_b = Ws.astype(np.float16)
    Wf_b = Wf.astype(np.float16)
    bxT = np.ascontiguousarray(bx.reshape(16, P).T)
    bsT = np.ascontiguousarray(bs.reshape(12, P).T)
    bfT = np.ascontiguousarray(bf.reshape(4, P).T)

    in_maps = []
    core_masks = []
    for c in range(NCORES):
        heaps = _core_heaps(c)
        valid = (heaps >= 0) & (heaps < N)
        M = np.zeros((NSLOT, IN_DIM), np.float32)
        M[valid] = inputs[N - 1 - heaps[valid]]
        xin = np.ascontiguousarray(M.T)
        mrow = valid[:NL3].astype(np.float32)
        cmask = np.ascontiguousarray(np.tile(mrow[None, :], (P, 1)))
        core_masks.append(valid)
        in_maps.append({
            "xin": xin.astype(np.float16), "wx": Wx_b, "ws": Ws_b,
            "wf": Wf_b, "bx": bxT, "bs": bsT, "bf": bfT, "cmask": cmask,
        })

    if _NC_CACHE is None:
        _NC_CACHE = _build_program()
    nc = _NC_CACHE

    res = run_bass_kernel_spmd(
        nc, in_maps, list(range(NCORES)),
        trace=bool(os.environ.get("BASS_TRACE")),
    )
    LAST_RESULT = res

    if TOP_ON_DEVICE:
        return np.ascontiguousarray(res.results[0]["out"])

    # host fallback: finish the 21-node top tree in numpy
    Hr = np.zeros((64, MEM), np.float32)
    Cr = np.zeros((64, MEM), np.float32)
    for c in range(NCORES):
        r = res.results[c]["roots"]  # [1024, 8]
        Hr[8 * c:8 * c + 8] = r[:MEM].T
        Cr[8 * c:8 * c + 8] = r[MEM:].T

    def np_step(Hc, Cc, X_par):
        sig = lambda v: 1.0 / (1.0 + np.exp(-v))
        chs = Hc.reshape(-1, 4, MEM).sum(1)
        iou = chs @ Ws + bs
        i = sig(X_par[:, :MEM] + iou[:, :MEM])
        o = sig(X_par[:, 2 * MEM:3 * MEM] + iou[:, MEM:2 * MEM])
        u = np.tanh(X_par[:, 3 * MEM:] + iou[:, 2 * MEM:])
        fx = np.repeat(X_par[:, MEM:2 * MEM], 4, axis=0)
        f = sig(Hc @ Wf + bf + fx)
        cc = i * u + (f.reshape(-1, 4, MEM) * Cc.reshape(-1, 4, MEM)).sum(1)
        return o * np.tanh(cc), cc

    X_all = inputs @ Wx + bx  # [N, 2048] (only 21 rows used)
    Xtop = lambda hs: X_all[N - 1 - np.array(hs)]
    h2, c2 = np_step(Hr, Cr, Xtop(range(5, 21)))
    h1, c1 = np_step(h2, c2, Xtop(range(1, 5)))
    h0, _ = np_step(h1, c1, Xtop([0]))
    return np.ascontiguousarray(h0.astype(np.float32))

